# revision 40
# baseline (speedup 1.0000x reference)
"""MoE gate (nn_MoEGate) Trainium2 Bass kernel.

Strategy (data-parallel over tokens, 8 cores):
  - Host: flatten x to [16384, 2048], scale by 2^5, split into fp16 hi/lo
    (lo pre-scaled by 2^11), shard tokens 8 ways and pre-shuffle each
    shard into the exact per-chunk SBUF layout [chunk][p][k][t] so every
    device DMA is a fully contiguous 2 MB stream (16 KB per partition).
    The gate weight is scaled by 2^10, split the same way, and PACKED
    into two 128-wide fp16 stationaries: w1 = [w_hi | 2^11*w_lo],
    w2 = [0 | w_hi].  Two fp16 matmuls per k-block then compute hi and
    lo partial products for 128 PSUM rows at once (rows 0:64 = hi,
    64:128 = lo); exact logits (scaled by 2^15) are recovered as
    hi + 2^-11 * lo after a PE transpose.  Max logit error ~4e-7 -
    bit-compatible with an f32 matmul for top-k purposes - at full
    16-bit PE throughput.
  - Device per core (2048 tokens, 4 chunks of 512, 16 tiles of 128):
      * 2x16 fp16 matmuls per chunk into PSUM [128, 512]
      * ACT copy -> SBUF, PE-transpose 128x128 blocks, DVE combine
        hi+2^-11*lo -> scaled logits [128 tokens, 64 experts]
      * ACT: exp(scale * logits) with accumulated per-token sum Z
      * ACT: Sign(logits - null) with accumulated sum -> m count
      * DVE: top-16 values+indices via max / max_index / match_replace
      * PE:  P_real partial = exp.T @ (1/Z) (cross-partition reduce)
      * outputs accumulate in SBUF, one DMA per output tensor at the end
  - Host: merge real top-k with the 64 identical null experts
    (all nulls share logit null_logit, so top-k of the 128-way concat is
    [reals >= null in sorted order] ++ [null 64, 65, ...]), renormalize
    weights, bincount, and assemble the aux loss.
"""

import os

import numpy as np

import concourse.mybir as mybir
from concourse import bacc
from concourse.bass_utils import run_bass_kernel_spmd
from concourse.masks import make_identity
from concourse.tile import TileContext

# Problem constants (fixed by the grading harness).
B, T, D = 4, 4096, 2048
E, NULL, K = 64, 64, 10
RHO = 0.5
N_CORES = 8
TOK = B * T              # 16384 tokens
TPC = TOK // N_CORES     # 2048 tokens per core
P = 128                  # tokens per tile (SBUF partitions)
CH = 512                 # tokens per PSUM chunk
NCH = TPC // CH          # 4 chunks per core
NTILES = TPC // P        # 16 tiles per core
KB = D // P              # 16 contraction blocks

SX = 2.0 ** 5            # x pre-scale (keeps fp16 lo parts normal)
SW = 2.0 ** 10           # w pre-scale
SC = SX * SW             # logits arrive scaled by 2^15
SLO = 2.0 ** 11          # lo-part pre-scale

f32 = mybir.dt.float32
f16 = mybir.dt.float16
u32 = mybir.dt.uint32

# Set by the last kernel() call when BASS_KERNEL_TRACE=1 (for test.py).
last_results = None


def _ensure_ntff_hook():
    """Register the axon NTFF profile hook if the antenv stub lacks it."""
    import sys
    import types

    try:
        from antenv.axon_hooks import get_axon_ntff_profile_hook  # noqa: F401
        return True
    except ImportError:
        pass
    try:
        import antenv
        from trn_agent_boot.trn_boot import _ntff_profile_via_ctypes

        hook = _ntff_profile_via_ctypes("/opt/axon/libaxon_pjrt.so")
        mod = types.ModuleType("antenv.axon_hooks")
        _state = {"hook": hook}
        mod.set_axon_ntff_profile_hook = lambda h: _state.__setitem__("hook", h)
        mod.get_axon_ntff_profile_hook = lambda: _state["hook"]
        sys.modules["antenv.axon_hooks"] = mod
        antenv.axon_hooks = mod
        return hook is not None
    except Exception:
        return False


def _build(null_logit: float, has_bias: bool):
    nc = bacc.Bacc(
        "TRN2",
        target_bir_lowering=False,
        debug=False,
        enable_asserts=True,
        num_devices=N_CORES,
    )
    xh = nc.dram_tensor("xh", [NCH, P, KB * CH], f16, kind="ExternalInput")
    xl = nc.dram_tensor("xl", [NCH, P, KB * CH], f16, kind="ExternalInput")
    w1 = nc.dram_tensor("w1", [P, KB * 2 * E], f16, kind="ExternalInput")
    w2 = nc.dram_tensor("w2", [P, KB * 2 * E], f16, kind="ExternalInput")
    if has_bias:
        bias_pack = nc.dram_tensor("bias_pack", [1, 2 * E], f16, kind="ExternalInput")
    out_v = nc.dram_tensor("out_v", [P, NTILES * 16], f32, kind="ExternalOutput")
    out_i = nc.dram_tensor("out_i", [P, NTILES * 16], u32, kind="ExternalOutput")
    out_z = nc.dram_tensor("out_z", [P, NTILES], f32, kind="ExternalOutput")
    out_p = nc.dram_tensor("out_p", [P, E], f32, kind="ExternalOutput")

    with TileContext(nc) as tc:
        with (
            tc.tile_pool(name="wpool", bufs=1) as wpool,
            tc.tile_pool(name="obuf", bufs=1) as obuf,
            tc.tile_pool(name="xpool", bufs=3) as xpool,
            tc.tile_pool(name="cpool", bufs=2) as cpool,
            tc.tile_pool(name="spool", bufs=3) as spool,
            tc.tile_pool(name="opool", bufs=4) as opool,
            tc.tile_pool(name="psmain", bufs=2, space="PSUM") as psmain,
            tc.tile_pool(name="pstr", bufs=4, space="PSUM") as pstr,
        ):
            w1_sb = wpool.tile([P, KB, 2 * E], f16, name="w1_sb")
            nc.gpsimd.dma_start(w1_sb, w1.rearrange("p (k e) -> p k e", k=KB))
            w2_sb = wpool.tile([P, KB, 2 * E], f16, name="w2_sb")
            nc.gpsimd.dma_start(w2_sb, w2.rearrange("p (k e) -> p k e", k=KB))
            # Combine matrix: transpose + (hi + 2^-11 lo) in one PE op.
            # Both products are exact powers of two, so this is bit-identical
            # to computing hi + 2^-11*lo on the vector engine.
            cmb_np = np.zeros((P, E), np.float32)
            cmb_np[np.arange(E), np.arange(E)] = 1.0
            cmb_np[E + np.arange(E), np.arange(E)] = np.float32(1.0 / SLO)
            cmb_dram = nc.inline_tensor(cmb_np, name="cmb_dram")
            cmb = wpool.tile([P, E], f32, name="cmb")
            nc.gpsimd.dma_start(cmb, cmb_dram[:, :])
            if has_bias:
                bias_sb = wpool.tile([1, 2 * E], f16, name="bias_sb")
                nc.gpsimd.dma_start(bias_sb, bias_pack[:, :])
                ones_sb = wpool.tile([1, CH], f16, name="ones_sb")
                nc.vector.memset(ones_sb, 1.0)
            accp = obuf.tile([P, E], f32, name="accp")
            nc.vector.memset(accp, 0.0)

            v_all = obuf.tile([P, NTILES * 16], f32, name="v_all")
            i_all = obuf.tile([P, NTILES * 16], u32, name="i_all")
            z_all = obuf.tile([P, NTILES], f32, name="z_all")

            lsbs = [None] * NCH

            def issue_chunk(c):
                # Split the loads so the first matmuls can start as soon as
                # the first slice of the chunk has landed (finest for chunk 0,
                # which gates kernel start).
                ksplit = 8 if c == 0 else 4
                kseg = KB // ksplit
                x1_sb = xpool.tile([P, KB * CH], f16, name="x1_sb")
                x2_sb = xpool.tile([P, KB * CH], f16, name="x2_sb")
                for s in range(ksplit):
                    sl = slice(s * kseg * CH, (s + 1) * kseg * CH)
                    nc.sync.dma_start(x1_sb[:, sl], xh[c, :, sl])
                    nc.sync.dma_start(x2_sb[:, sl], xl[c, :, sl])

                ps_main = psmain.tile([P, CH], f32, name="ps_main")
                for k in range(KB):
                    nc.tensor.matmul(
                        ps_main, w1_sb[:, k, :], x1_sb[:, k * CH:(k + 1) * CH],
                        start=(k == 0), stop=False,
                    )
                    nc.tensor.matmul(
                        ps_main, w2_sb[:, k, :], x2_sb[:, k * CH:(k + 1) * CH],
                        start=False, stop=(k == KB - 1 and not has_bias),
                    )
                if has_bias:
                    nc.tensor.matmul(ps_main, bias_sb, ones_sb, start=False, stop=True)

                lsb = cpool.tile([P, CH], f32, name="lsb")
                nc.scalar.copy(lsb, ps_main)
                lsbs[c] = lsb

            def process_chunk(c):
                lsb = lsbs[c]
                for tt in range(NCH):
                    t = c * NCH + tt
                    # Transpose + hi/lo combine in one PE op (scale SC=2^15).
                    tps = pstr.tile([P, E], f32, name="tps")
                    nc.tensor.matmul(tps, lsb[:, tt * P:(tt + 1) * P], cmb,
                                     start=True, stop=True)

                    logits_sb = spool.tile([P, E], f32, name="logits_sb")
                    nc.scalar.copy(logits_sb, tps)

                    exp_sb = spool.tile([P, E], f32, name="exp_sb")
                    nc.scalar.activation(
                        exp_sb, logits_sb, mybir.ActivationFunctionType.Exp,
                        scale=1.0 / SC, accum_out=z_all[:, t:t + 1],
                    )
                    vs = v_all[:, t * 16:t * 16 + 8]
                    vs2 = v_all[:, t * 16 + 8:t * 16 + 16]
                    work2 = spool.tile([P, E], f32, name="work2")
                    nc.vector.max(out=vs, in_=logits_sb)
                    nc.vector.max_index(out=i_all[:, t * 16:t * 16 + 8],
                                        in_max=vs, in_values=logits_sb)
                    nc.vector.match_replace(out=work2, in_to_replace=vs,
                                            in_values=logits_sb, imm_value=-1e30)
                    nc.vector.max(out=vs2, in_=work2)
                    nc.vector.max_index(out=i_all[:, t * 16 + 8:t * 16 + 16],
                                        in_max=vs2, in_values=work2)

                    rz = opool.tile([P, 1], f32, name="rz")
                    nc.vector.reciprocal(rz, z_all[:, t:t + 1])
                    # P_real partial accumulation: accp += exp * (1/Z)
                    nc.vector.scalar_tensor_tensor(
                        out=accp,
                        in0=exp_sb,
                        scalar=rz,
                        in1=accp,
                        op0=mybir.AluOpType.mult,
                        op1=mybir.AluOpType.add,
                    )

            # Software pipeline: keep the PE stream one chunk ahead of the
            # transpose/combine work so cross-engine latency never stalls it.
            issue_chunk(0)
            for c in range(1, NCH):
                issue_chunk(c)
                process_chunk(c - 1)
            process_chunk(NCH - 1)

            nc.gpsimd.dma_start(out_p[:, :], accp)
            nc.gpsimd.dma_start(out_v[:, :], v_all)
            nc.gpsimd.dma_start(out_i[:, :], i_all)
            nc.gpsimd.dma_start(out_z[:, :], z_all)
    nc.finalize()
    return nc


def _split_f16(a32):
    """Split f32 array into fp16 hi + fp16 (2^11 * lo)."""
    hi = a32.astype(np.float16)
    lo = ((a32 - hi.astype(np.float32)) * np.float32(SLO)).astype(np.float16)
    return hi, lo


def _chunk_layout(shard_f16):
    """[TPC, D] fp16 -> [NCH, P, KB*CH] in the SBUF DMA layout."""
    r = shard_f16.reshape(NCH, CH, KB, P)         # (c, t, k, p)
    return np.ascontiguousarray(r.transpose(0, 3, 2, 1)).reshape(NCH, P, KB * CH)


def kernel(x, gate_w, logit_bias, null_logit):
    global last_results
    x = np.asarray(x, dtype=np.float32)
    gate_w = np.asarray(gate_w, dtype=np.float32)
    logit_bias = np.asarray(logit_bias, dtype=np.float32).reshape(E)
    null_f = float(np.asarray(null_logit))
    has_bias = bool(np.any(logit_bias != 0.0))

    xs = x.reshape(TOK, D) * np.float32(SX)
    xh_full, xl_full = _split_f16(xs)

    ws = gate_w.T * np.float32(SW)        # [D, E]
    wh_h, wl_h = _split_f16(ws)
    w1 = np.zeros((D, 2 * E), np.float16)
    w1[:, :E] = wh_h
    w1[:, E:] = wl_h
    w2 = np.zeros((D, 2 * E), np.float16)
    w2[:, E:] = wh_h
    # [D, 2E] -> [P, KB*2E] with d = k*128 + p
    wpk1 = np.ascontiguousarray(
        w1.reshape(KB, P, 2 * E).transpose(1, 0, 2)).reshape(P, KB * 2 * E)
    wpk2 = np.ascontiguousarray(
        w2.reshape(KB, P, 2 * E).transpose(1, 0, 2)).reshape(P, KB * 2 * E)

    nc = _build(null_f, has_bias)
    in_maps = []
    for c in range(N_CORES):
        im = {
            "xh": _chunk_layout(xh_full[c * TPC:(c + 1) * TPC]),
            "xl": _chunk_layout(xl_full[c * TPC:(c + 1) * TPC]),
            "w1": wpk1,
            "w2": wpk2,
        }
        if has_bias:
            bs = logit_bias.astype(np.float64) * SC
            bh = bs.astype(np.float16)
            bl = ((bs - bh.astype(np.float64)) * SLO).astype(np.float16)
            bp = np.zeros((1, 2 * E), np.float16)
            bp[0, :E] = bh
            bp[0, E:] = bl
            im["bias_pack"] = bp
        in_maps.append(im)

    trace = os.environ.get("BASS_KERNEL_TRACE", "0") == "1"
    if trace:
        trace = _ensure_ntff_hook()
        try:
            # Artifact upload has no bucket in this container; neuter it.
            import concourse.bass_utils as _bu

            _bu.upload_artifacts = lambda tmpdir: tmpdir
        except Exception:
            pass
    res = run_bass_kernel_spmd(
        nc, in_maps, core_ids=list(range(N_CORES)), trace=trace
    )
    last_results = res
    rs = res.results

    def untile(a, width):
        # [P, NTILES*width] -> [TPC, width] with token = t*128 + p
        return a.reshape(P, NTILES, width).transpose(1, 0, 2).reshape(TPC, width)

    v = np.concatenate([untile(r["out_v"], 16) for r in rs])[:, :K]
    i = np.concatenate([untile(r["out_i"], 16) for r in rs])[:, :K]
    z = np.concatenate([untile(r["out_z"], 1) for r in rs]).reshape(-1).astype(np.float64)
    p_partial = np.stack(
        [r["out_p"].astype(np.float64).sum(axis=0) for r in rs]
    )  # [N_CORES, E]

    # Host-side merge of real top-k with the identical-valued null experts.
    # Slot j holds a real expert iff the j-th largest real logit >= null
    # (ties break in favour of reals: lower index wins in the reference).
    C = float(NULL) * float(np.exp(np.float64(null_f)))
    null_s = np.float64(null_f) * SC
    valid = v.astype(np.float64) >= null_s
    m_int = valid.sum(-1, keepdims=True).astype(np.float64)   # used when < K
    jj = np.arange(K, dtype=np.float64)[None, :]
    nullidx = (E + jj - m_int)
    idx = np.where(valid, i.astype(np.int64), nullidx.astype(np.int64))
    ev = np.exp(v.astype(np.float64) / SC)   # v holds 2^15-scaled top-10 logits
    evmask = np.where(valid, ev, 0.0)
    wsum = evmask.sum(-1)
    zfull = z + C
    denom = np.maximum(wsum, 1e-6 * zfull)
    topk_w = (evmask / denom[:, None]).astype(np.float32)
    is_null = ~valid

    P_real = p_partial.sum(0) / TOK
    counts = np.bincount(idx[valid], minlength=E).astype(np.float64)[:E]
    f_real = counts / np.clip(counts.sum(), 1e-6, None)
    L_bal = E * float((f_real * P_real).sum())
    lse = np.log(zfull)
    L_z = float(np.mean(lse ** 2))
    null_rate = float(is_null.mean())
    L_null = (null_rate - RHO) ** 2
    aux = np.float32(0.02 * L_bal + 0.001 * L_z + 0.01 * L_null)

    return (
        idx.reshape(B, T, K).astype(np.int32),
        topk_w.reshape(B, T, K),
        is_null.reshape(B, T, K),
        aux,
    )


# revision 41
# speedup vs baseline: 1.0823x; 1.0823x over previous
"""MoE gate (nn_MoEGate) Trainium2 Bass kernel.

Strategy (data-parallel over tokens, 8 cores):
  - Host: flatten x to [16384, 2048], scale by 2^5, split into fp16 hi/lo
    (lo pre-scaled by 2^11), shard tokens 8 ways and pre-shuffle each
    shard into the exact per-chunk SBUF layout [chunk][p][k][t] so every
    device DMA is a fully contiguous 2 MB stream (16 KB per partition).
    The gate weight is scaled by 2^10, split the same way, and PACKED
    into two 128-wide fp16 stationaries: w1 = [w_hi | 2^11*w_lo],
    w2 = [0 | w_hi].  Two fp16 matmuls per k-block then compute hi and
    lo partial products for 128 PSUM rows at once (rows 0:64 = hi,
    64:128 = lo); exact logits (scaled by 2^15) are recovered as
    hi + 2^-11 * lo after a PE transpose.  Max logit error ~4e-7 -
    bit-compatible with an f32 matmul for top-k purposes - at full
    16-bit PE throughput.
  - Device per core (2048 tokens, 4 chunks of 512, 16 tiles of 128):
      * 2x16 fp16 matmuls per chunk into PSUM [128, 512]
      * ACT copy -> SBUF, PE-transpose 128x128 blocks, DVE combine
        hi+2^-11*lo -> scaled logits [128 tokens, 64 experts]
      * ACT: exp(scale * logits) with accumulated per-token sum Z
      * ACT: Sign(logits - null) with accumulated sum -> m count
      * DVE: top-16 values+indices via max / max_index / match_replace
      * PE:  P_real partial = exp.T @ (1/Z) (cross-partition reduce)
      * outputs accumulate in SBUF, one DMA per output tensor at the end
  - Host: merge real top-k with the 64 identical null experts
    (all nulls share logit null_logit, so top-k of the 128-way concat is
    [reals >= null in sorted order] ++ [null 64, 65, ...]), renormalize
    weights, bincount, and assemble the aux loss.
"""

import os

import numpy as np

import concourse.mybir as mybir
from concourse import bacc
from concourse.bass_utils import run_bass_kernel_spmd
from concourse.masks import make_identity
from concourse.tile import TileContext

# Problem constants (fixed by the grading harness).
B, T, D = 4, 4096, 2048
E, NULL, K = 64, 64, 10
RHO = 0.5
N_CORES = 8
TOK = B * T              # 16384 tokens
TPC = TOK // N_CORES     # 2048 tokens per core
P = 128                  # tokens per tile (SBUF partitions)
CH = 512                 # tokens per PSUM chunk
NCH = TPC // CH          # 4 chunks per core
NTILES = TPC // P        # 16 tiles per core
KB = D // P              # 16 contraction blocks

SX = 2.0 ** 5            # x pre-scale (keeps fp16 lo parts normal)
SW = 2.0 ** 10           # w pre-scale
SC = SX * SW             # logits arrive scaled by 2^15
SLO = 2.0 ** 11          # lo-part pre-scale

f32 = mybir.dt.float32
f16 = mybir.dt.float16
u32 = mybir.dt.uint32

# Set by the last kernel() call when BASS_KERNEL_TRACE=1 (for test.py).
last_results = None


def _ensure_ntff_hook():
    """Register the axon NTFF profile hook if the antenv stub lacks it."""
    import sys
    import types

    try:
        from antenv.axon_hooks import get_axon_ntff_profile_hook  # noqa: F401
        return True
    except ImportError:
        pass
    try:
        import antenv
        from trn_agent_boot.trn_boot import _ntff_profile_via_ctypes

        hook = _ntff_profile_via_ctypes("/opt/axon/libaxon_pjrt.so")
        mod = types.ModuleType("antenv.axon_hooks")
        _state = {"hook": hook}
        mod.set_axon_ntff_profile_hook = lambda h: _state.__setitem__("hook", h)
        mod.get_axon_ntff_profile_hook = lambda: _state["hook"]
        sys.modules["antenv.axon_hooks"] = mod
        antenv.axon_hooks = mod
        return hook is not None
    except Exception:
        return False


def _build(null_logit: float, has_bias: bool):
    nc = bacc.Bacc(
        "TRN2",
        target_bir_lowering=False,
        debug=False,
        enable_asserts=True,
        num_devices=N_CORES,
    )
    xh = nc.dram_tensor("xh", [NCH, P, KB * CH], f16, kind="ExternalInput")
    xl = nc.dram_tensor("xl", [NCH, P, KB * CH], f16, kind="ExternalInput")
    w1 = nc.dram_tensor("w1", [P, KB * 2 * E], f16, kind="ExternalInput")
    w2 = nc.dram_tensor("w2", [P, KB * 2 * E], f16, kind="ExternalInput")
    if has_bias:
        bias_pack = nc.dram_tensor("bias_pack", [1, 2 * E], f16, kind="ExternalInput")
    out_v = nc.dram_tensor("out_v", [P, NTILES * 16], f32, kind="ExternalOutput")
    out_i = nc.dram_tensor("out_i", [P, NTILES * 16], u32, kind="ExternalOutput")
    out_z = nc.dram_tensor("out_z", [P, NTILES], f32, kind="ExternalOutput")
    out_p = nc.dram_tensor("out_p", [P, E], f32, kind="ExternalOutput")

    with TileContext(nc) as tc:
        with (
            tc.tile_pool(name="wpool", bufs=1) as wpool,
            tc.tile_pool(name="obuf", bufs=1) as obuf,
            tc.tile_pool(name="xpool", bufs=3) as xpool,
            tc.tile_pool(name="cpool", bufs=2) as cpool,
            tc.tile_pool(name="spool", bufs=3) as spool,
            tc.tile_pool(name="opool", bufs=4) as opool,
            tc.tile_pool(name="psmain", bufs=2, space="PSUM") as psmain,
            tc.tile_pool(name="pstr", bufs=4, space="PSUM") as pstr,
        ):
            w1_sb = wpool.tile([P, KB, 2 * E], f16, name="w1_sb")
            nc.gpsimd.dma_start(w1_sb, w1.rearrange("p (k e) -> p k e", k=KB))
            w2_sb = wpool.tile([P, KB, 2 * E], f16, name="w2_sb")
            nc.gpsimd.dma_start(w2_sb, w2.rearrange("p (k e) -> p k e", k=KB))
            # Combine matrix: transpose + (hi + 2^-11 lo) in one PE op.
            # Both products are exact powers of two, so this is bit-identical
            # to computing hi + 2^-11*lo on the vector engine.
            cmb_np = np.zeros((P, E), np.float32)
            cmb_np[np.arange(E), np.arange(E)] = 1.0
            cmb_np[E + np.arange(E), np.arange(E)] = np.float32(1.0 / SLO)
            cmb_dram = nc.inline_tensor(cmb_np, name="cmb_dram")
            cmb = wpool.tile([P, E], f32, name="cmb")
            nc.gpsimd.dma_start(cmb, cmb_dram[:, :])
            if has_bias:
                bias_sb = wpool.tile([1, 2 * E], f16, name="bias_sb")
                nc.gpsimd.dma_start(bias_sb, bias_pack[:, :])
                ones_sb = wpool.tile([1, CH], f16, name="ones_sb")
                nc.vector.memset(ones_sb, 1.0)
            accp = obuf.tile([P, E], f32, name="accp")
            nc.vector.memset(accp, 0.0)

            v_all = obuf.tile([P, NTILES * 16], f32, name="v_all")
            i_all = obuf.tile([P, NTILES * 16], u32, name="i_all")
            z_all = obuf.tile([P, NTILES], f32, name="z_all")

            lsbs = [None] * NCH

            def issue_chunk(c):
                # Split the loads so the first matmuls can start as soon as
                # the first slice of the chunk has landed (finest for chunk 0,
                # which gates kernel start).
                ksplit = 8 if c == 0 else 4
                kseg = KB // ksplit
                x1_sb = xpool.tile([P, KB * CH], f16, name="x1_sb")
                x2_sb = xpool.tile([P, KB * CH], f16, name="x2_sb")
                for s in range(ksplit):
                    sl = slice(s * kseg * CH, (s + 1) * kseg * CH)
                    nc.sync.dma_start(x1_sb[:, sl], xh[c, :, sl])
                    nc.sync.dma_start(x2_sb[:, sl], xl[c, :, sl])

                ps_main = psmain.tile([P, CH], f32, name="ps_main")
                for k in range(KB):
                    nc.tensor.matmul(
                        ps_main, w1_sb[:, k, :], x1_sb[:, k * CH:(k + 1) * CH],
                        start=(k == 0), stop=False,
                    )
                    nc.tensor.matmul(
                        ps_main, w2_sb[:, k, :], x2_sb[:, k * CH:(k + 1) * CH],
                        start=False, stop=(k == KB - 1 and not has_bias),
                    )
                if has_bias:
                    nc.tensor.matmul(ps_main, bias_sb, ones_sb, start=False, stop=True)

                lsb = cpool.tile([P, CH], f32, name="lsb")
                nc.scalar.copy(lsb, ps_main)
                lsbs[c] = lsb

            def process_chunk(c):
                lsb = lsbs[c]
                for tt in range(NCH):
                    t = c * NCH + tt
                    # Transpose + hi/lo combine in one PE op (scale SC=2^15).
                    tps = pstr.tile([P, E], f32, name="tps")
                    nc.tensor.matmul(tps, lsb[:, tt * P:(tt + 1) * P], cmb,
                                     start=True, stop=True)

                    logits_sb = spool.tile([P, E], f32, name="logits_sb")
                    nc.scalar.copy(logits_sb, tps)

                    exp_sb = spool.tile([P, E], f32, name="exp_sb")
                    nc.scalar.activation(
                        exp_sb, logits_sb, mybir.ActivationFunctionType.Exp,
                        scale=1.0 / SC, accum_out=z_all[:, t:t + 1],
                    )
                    vs = v_all[:, t * 16:t * 16 + 8]
                    vs2 = v_all[:, t * 16 + 8:t * 16 + 16]
                    work2 = spool.tile([P, E], f32, name="work2")
                    nc.vector.max(out=vs, in_=logits_sb)
                    nc.vector.max_index(out=i_all[:, t * 16:t * 16 + 8],
                                        in_max=vs, in_values=logits_sb)
                    nc.vector.match_replace(out=work2, in_to_replace=vs,
                                            in_values=logits_sb, imm_value=-1e30)
                    nc.vector.max(out=vs2, in_=work2)
                    nc.vector.max_index(out=i_all[:, t * 16 + 8:t * 16 + 16],
                                        in_max=vs2, in_values=work2)

                    rz = opool.tile([P, 1], f32, name="rz")
                    nc.vector.reciprocal(rz, z_all[:, t:t + 1])
                    # P_real partial accumulation: accp += exp * (1/Z)
                    nc.vector.scalar_tensor_tensor(
                        out=accp,
                        in0=exp_sb,
                        scalar=rz,
                        in1=accp,
                        op0=mybir.AluOpType.mult,
                        op1=mybir.AluOpType.add,
                    )

            # Software pipeline: keep the PE stream one chunk ahead of the
            # transpose/combine work so cross-engine latency never stalls it.
            issue_chunk(0)
            for c in range(1, NCH):
                issue_chunk(c)
                process_chunk(c - 1)
                if c == NCH - 1:
                    # First-half output flush while the PE is still busy.
                    HT = (NTILES // 2) * 16
                    nc.sync.dma_start(out_v[:, :HT], v_all[:, :HT])
                    nc.sync.dma_start(out_i[:, :HT], i_all[:, :HT])
                    nc.sync.dma_start(out_z[:, :NTILES // 2], z_all[:, :NTILES // 2])
            process_chunk(NCH - 1)

            # Second-half output flush (first half went out after chunk 2).
            HT = (NTILES // 2) * 16
            nc.sync.dma_start(out_v[:, HT:], v_all[:, HT:])
            nc.sync.dma_start(out_i[:, HT:], i_all[:, HT:])
            nc.sync.dma_start(out_z[:, NTILES // 2:], z_all[:, NTILES // 2:])
            nc.sync.dma_start(out_p[:, :], accp)
    nc.finalize()
    return nc


def _split_f16(a32):
    """Split f32 array into fp16 hi + fp16 (2^11 * lo)."""
    hi = a32.astype(np.float16)
    lo = ((a32 - hi.astype(np.float32)) * np.float32(SLO)).astype(np.float16)
    return hi, lo


def _chunk_layout(shard_f16):
    """[TPC, D] fp16 -> [NCH, P, KB*CH] in the SBUF DMA layout."""
    r = shard_f16.reshape(NCH, CH, KB, P)         # (c, t, k, p)
    return np.ascontiguousarray(r.transpose(0, 3, 2, 1)).reshape(NCH, P, KB * CH)


def kernel(x, gate_w, logit_bias, null_logit):
    global last_results
    x = np.asarray(x, dtype=np.float32)
    gate_w = np.asarray(gate_w, dtype=np.float32)
    logit_bias = np.asarray(logit_bias, dtype=np.float32).reshape(E)
    null_f = float(np.asarray(null_logit))
    has_bias = bool(np.any(logit_bias != 0.0))

    xs = x.reshape(TOK, D) * np.float32(SX)
    xh_full, xl_full = _split_f16(xs)

    ws = gate_w.T * np.float32(SW)        # [D, E]
    wh_h, wl_h = _split_f16(ws)
    w1 = np.zeros((D, 2 * E), np.float16)
    w1[:, :E] = wh_h
    w1[:, E:] = wl_h
    w2 = np.zeros((D, 2 * E), np.float16)
    w2[:, E:] = wh_h
    # [D, 2E] -> [P, KB*2E] with d = k*128 + p
    wpk1 = np.ascontiguousarray(
        w1.reshape(KB, P, 2 * E).transpose(1, 0, 2)).reshape(P, KB * 2 * E)
    wpk2 = np.ascontiguousarray(
        w2.reshape(KB, P, 2 * E).transpose(1, 0, 2)).reshape(P, KB * 2 * E)

    nc = _build(null_f, has_bias)
    in_maps = []
    for c in range(N_CORES):
        im = {
            "xh": _chunk_layout(xh_full[c * TPC:(c + 1) * TPC]),
            "xl": _chunk_layout(xl_full[c * TPC:(c + 1) * TPC]),
            "w1": wpk1,
            "w2": wpk2,
        }
        if has_bias:
            bs = logit_bias.astype(np.float64) * SC
            bh = bs.astype(np.float16)
            bl = ((bs - bh.astype(np.float64)) * SLO).astype(np.float16)
            bp = np.zeros((1, 2 * E), np.float16)
            bp[0, :E] = bh
            bp[0, E:] = bl
            im["bias_pack"] = bp
        in_maps.append(im)

    trace = os.environ.get("BASS_KERNEL_TRACE", "0") == "1"
    if trace:
        trace = _ensure_ntff_hook()
        try:
            # Artifact upload has no bucket in this container; neuter it.
            import concourse.bass_utils as _bu

            _bu.upload_artifacts = lambda tmpdir: tmpdir
        except Exception:
            pass
    res = run_bass_kernel_spmd(
        nc, in_maps, core_ids=list(range(N_CORES)), trace=trace
    )
    last_results = res
    rs = res.results

    def untile(a, width):
        # [P, NTILES*width] -> [TPC, width] with token = t*128 + p
        return a.reshape(P, NTILES, width).transpose(1, 0, 2).reshape(TPC, width)

    v = np.concatenate([untile(r["out_v"], 16) for r in rs])[:, :K]
    i = np.concatenate([untile(r["out_i"], 16) for r in rs])[:, :K]
    z = np.concatenate([untile(r["out_z"], 1) for r in rs]).reshape(-1).astype(np.float64)
    p_partial = np.stack(
        [r["out_p"].astype(np.float64).sum(axis=0) for r in rs]
    )  # [N_CORES, E]

    # Host-side merge of real top-k with the identical-valued null experts.
    # Slot j holds a real expert iff the j-th largest real logit >= null
    # (ties break in favour of reals: lower index wins in the reference).
    C = float(NULL) * float(np.exp(np.float64(null_f)))
    null_s = np.float64(null_f) * SC
    valid = v.astype(np.float64) >= null_s
    m_int = valid.sum(-1, keepdims=True).astype(np.float64)   # used when < K
    jj = np.arange(K, dtype=np.float64)[None, :]
    nullidx = (E + jj - m_int)
    idx = np.where(valid, i.astype(np.int64), nullidx.astype(np.int64))
    ev = np.exp(v.astype(np.float64) / SC)   # v holds 2^15-scaled top-10 logits
    evmask = np.where(valid, ev, 0.0)
    wsum = evmask.sum(-1)
    zfull = z + C
    denom = np.maximum(wsum, 1e-6 * zfull)
    topk_w = (evmask / denom[:, None]).astype(np.float32)
    is_null = ~valid

    P_real = p_partial.sum(0) / TOK
    counts = np.bincount(idx[valid], minlength=E).astype(np.float64)[:E]
    f_real = counts / np.clip(counts.sum(), 1e-6, None)
    L_bal = E * float((f_real * P_real).sum())
    lse = np.log(zfull)
    L_z = float(np.mean(lse ** 2))
    null_rate = float(is_null.mean())
    L_null = (null_rate - RHO) ** 2
    aux = np.float32(0.02 * L_bal + 0.001 * L_z + 0.01 * L_null)

    return (
        idx.reshape(B, T, K).astype(np.int32),
        topk_w.reshape(B, T, K),
        is_null.reshape(B, T, K),
        aux,
    )
